# revision 12
# baseline (speedup 1.0000x reference)
"""DeltaNet-style gated linear attention forward on 8 Trainium2 NeuronCores.

Sharding: core c = (batch b = c//4, sequence quarter r = c%4). Each core
projects q/k/v/gate for its 512 rows (all 16 heads), runs chunked linear
attention in quarter-local unscaled coordinates, exchanges per-quarter state
summaries via a small AllGather, then applies the inter-quarter state, output
projection, residual and LayerNorm for its own rows.

Math (per batch, head), matching the reference scan: with
b_i = prod_{j<=i} f_j (cumprod from quarter start), q~_i = phi_i * b_i,
k~_j = phi_j / b_j, the output row i inside a quarter is
  out_i = q~_i (S_start + U_i) / max(q~_i . (m_start + mU_i), eps)
where U_i = sum_{j<=i, same quarter} k~_j v_j^T accumulates unscaled and
(S_start, m_start) is the true state entering the quarter, combined from the
peer quarters' summaries (A_q, A_q * U_q) after an AllGather.

Precision: projections and attention matmuls run in bf16 (inputs) with fp32
PSUM accumulation; the gate/decay chain, state accumulation, normalization
and LayerNorm stay fp32.
"""

import numpy as np
import ml_dtypes

import bass_rust
import concourse.bass as bass
import concourse.mybir as mybir
import concourse.tile as tile
from concourse.bass_utils import run_bass_kernel_spmd

dt = mybir.dt
AF = mybir.ActivationFunctionType
ALU = mybir.AluOpType

B, T, C, H, D = 2, 2048, 1024, 16, 64
NCORE = 8
QT = T // 4          # rows per core
L = 128              # chunk length
NCH = QT // L        # chunks per core
KT = C // 128        # contraction tiles
SE = D + 1           # state row width (S | m)
ROPE_BASE = 10000.0
EPS = 1e-6
LN_EPS = 1e-5
G_CLAMP = -30.0      # per-chunk cumsum floor (defensive; inert for real data)
SPLIT_WAITS = True   # walrus here takes <=1 sem wait per instruction


# ---------------------------------------------------------------- walrus shim
def _split_multi_waits(nc):
    ctr = 0
    for fn in nc.m.functions:
        for bb in fn.blocks:
            out = []
            for ins in bb.instructions:
                si = ins.sync_info
                if si is not None and si.on_wait and len(si.on_wait) > 1:
                    waits = list(si.on_wait)
                    for w in waits[:-1]:
                        ctr += 1
                        nop = mybir.InstNoOp(name=f"WS-{ctr}", ins=[], outs=[])
                        nop.engine = ins.engine
                        nop.sync_info = bass_rust.SyncInfo(on_wait=[w], on_update=[])
                        nop.debug = ins.debug
                        out.append(nop)
                    si.on_wait = [waits[-1]]
                out.append(ins)
            bb.instructions[:] = out
    return ctr


def _register_const(nc, value, dtype=dt.float32):
    t = nc.alloc_sbuf_tensor(f"uconst-{dtype.name}-{value}", [128, 1], dtype)
    nc.gpsimd.memset(t.ap(), value)
    nc.const_aps.aps[(dtype, value)] = t.ap()


# ------------------------------------------------------------------- builder
def build(has_mask=False, has_ln=False):
    nc = bass.Bass(target_bir_lowering=False, debug=False)
    _register_const(nc, float(LN_EPS))
    nc.all_engine_barrier()

    f32 = dt.float32
    bf16 = dt.bfloat16
    P = {}

    def param(name, shape, dtype=f32, out=False):
        P[name] = nc.declare_dram_parameter(name, list(shape), dtype, isOutput=out)
        return P[name]

    param("xT", (128, KT, QT))                  # x rows^T (fp32, gate path)
    param("xTb", (128, KT, QT), bf16)           # x rows^T (bf16, qkv path)
    param("wqkv", (128, KT, 3 * C), bf16)       # [Wq.T|Wk.T|Wv.T] k-tiled
    param("wg", (128, KT, H))                   # Wg.T k-tiled (fp32)
    param("wo", (128, KT, C), bf16)             # Wo.T k-tiled
    param("xrows", (QT, C))                     # residual rows
    param("bo2", (1, C))                        # output bias row
    param("ropec", (128, NCH, 128))             # [cos|cos|-sin|+sin] per chunk
    param("triu", (128, 128))                   # j<=i ones (cumsum + causal)
    param("eye", (128, 128), bf16)              # PE transpose identity (bf16)
    param("eyef", (128, 128))                   # PE transpose identity (fp32)
    param("onesrow", (1, 128))
    param("bgt", (128, H))                      # gate bias, broadcast
    param("sel", (128, 4))                      # quarter-combine select (q < r)
    param("isel", (128, 4))                     # 1 - sel
    if has_mask:
        param("mkc", (128, NCH))
        param("mki", (128, NCH))
    if has_ln:
        param("lnw", (128, C))
        param("lnb", (128, C))
    param("y", (QT, C), out=True)

    one_ap = nc.const_aps.aps[(f32, 1.0)]

    import contextlib
    with tile.TileContext(nc) as tc, contextlib.ExitStack() as outer:
        keep = outer.enter_context(tc.tile_pool(name="keep", bufs=1))
        qtp = outer.enter_context(tc.tile_pool(name="qtp", bufs=NCH))
        nip = outer.enter_context(tc.tile_pool(name="nip", bufs=NCH))
        usp = outer.enter_context(tc.tile_pool(name="usp", bufs=NCH))
        dram = outer.enter_context(tc.tile_pool(name="dram", bufs=1, space="DRAM"))

        # constants
        triu_sb = keep.tile([128, 128], f32, name="triu_sb")
        eye_sb = keep.tile([128, 128], bf16, name="eye_sb")
        eyef_sb = keep.tile([128, 128], f32, name="eyef_sb")
        ones_row = keep.tile([1, 128], f32, name="ones_row")
        bgt_sb = keep.tile([128, H], f32, name="bgt_sb")
        rope_sb = keep.tile([128, NCH, 128], f32, name="rope_sb")
        sel_sb = keep.tile([128, 4], f32, name="sel_sb")
        isel_sb = keep.tile([128, 4], f32, name="isel_sb")
        for t_, p_ in ((triu_sb, "triu"), (eye_sb, "eye"), (eyef_sb, "eyef"),
                       (ones_row, "onesrow"), (bgt_sb, "bgt"),
                       (rope_sb, "ropec"), (sel_sb, "sel"), (isel_sb, "isel")):
            nc.sync.dma_start(t_[:], P[p_][:])
        if has_mask:
            mkc_sb = keep.tile([128, NCH], f32, name="mkc_sb")
            mki_sb = keep.tile([128, NCH], f32, name="mki_sb")
            nc.sync.dma_start(mkc_sb[:], P["mkc"][:])
            nc.sync.dma_start(mki_sb[:], P["mki"][:])

        qt_tiles, ni_tiles, u_tiles = [], [], []
        brun = keep.tile([128, H], f32, name="brun")
        binvr = keep.tile([128, H], f32, name="binvr")
        nc.vector.tensor_copy(brun[:], one_ap.to_broadcast([128, H]))
        nc.vector.tensor_copy(binvr[:], one_ap.to_broadcast([128, H]))

        # ---------------------------------------------------------- phase 1
        with contextlib.ExitStack() as ph1:
            wp = ph1.enter_context(tc.tile_pool(name="wp", bufs=1))
            work = ph1.enter_context(tc.tile_pool(name="work", bufs=2))
            small = ph1.enter_context(tc.tile_pool(name="small", bufs=2))
            pp = ph1.enter_context(tc.tile_pool(name="pp", bufs=1, space="PSUM"))
            sp = ph1.enter_context(tc.tile_pool(name="sp", bufs=1, space="PSUM"))
            tp = ph1.enter_context(tc.tile_pool(name="tp", bufs=2, space="PSUM"))
            ap_ = ph1.enter_context(tc.tile_pool(name="ap", bufs=1, space="PSUM"))
            up = ph1.enter_context(tc.tile_pool(name="up", bufs=1, space="PSUM"))

            wb_sb = wp.tile([128, KT, 3 * C], bf16, name="wb_sb")
            wg_sb = wp.tile([128, KT, H], f32, name="wg_sb")
            xt_sb = wp.tile([128, KT, QT], f32, name="xt_sb")
            xtb_sb = wp.tile([128, KT, QT], bf16, name="xtb_sb")
            for kt in range(KT):
                nc.sync.dma_start(wb_sb[:, kt, :], P["wqkv"][:, kt, :])
                nc.sync.dma_start(xt_sb[:, kt, :], P["xT"][:, kt, :])
                nc.sync.dma_start(xtb_sb[:, kt, :], P["xTb"][:, kt, :])
            nc.sync.dma_start(wg_sb[:], P["wg"][:])

            for ch in range(NCH):
                tsl = bass.ts(ch, L)

                # ---- gate: f = clip(sigmoid(z + bg)); logf; cumsums
                fz = sp.tile([128, H], f32, name="fz", tag="sps")
                for kt in range(KT):
                    nc.tensor.matmul(fz[:], xt_sb[:, kt, tsl],
                                     wg_sb[:, kt, :],
                                     start=(kt == 0), stop=(kt == KT - 1))
                logf = small.tile([128, H], f32, name="logf", tag="logf")
                nc.vector.tensor_add(logf[:], fz[:], bgt_sb[:])
                nc.scalar.activation(logf[:], logf[:], AF.Sigmoid)
                nc.vector.tensor_scalar(logf[:], logf[:], 0.999, 0.01,
                                        ALU.min, ALU.max)
                if has_mask:
                    nc.vector.tensor_mul(
                        logf[:], logf[:],
                        mkc_sb[:, ch:ch + 1].to_broadcast([128, H]))
                    nc.vector.tensor_add(
                        logf[:], logf[:],
                        mki_sb[:, ch:ch + 1].to_broadcast([128, H]))
                nc.scalar.activation(logf[:], logf[:], AF.Ln)

                gps = sp.tile([128, H], f32, name="gps", tag="sps")
                nc.tensor.matmul(gps[:], triu_sb[:], logf[:],
                                 start=True, stop=True)
                glp = sp.tile([1, H], f32, name="glp", tag="sps")
                nc.tensor.matmul(glp[:], one_ap[:, 0:1], logf[:],
                                 start=True, stop=True)

                a_c = small.tile([128, H], f32, name="a_c", tag="a_c")
                nc.vector.tensor_scalar_max(a_c[:], gps[:], G_CLAMP)
                ainv = small.tile([128, H], f32, name="ainv", tag="ainv")
                nc.scalar.activation(ainv[:], a_c[:], AF.Exp, scale=-1.0)
                nc.scalar.activation(a_c[:], a_c[:], AF.Exp)

                b16 = small.tile([128, H], bf16, name="b16", tag="b16")
                nc.vector.tensor_mul(b16[:], a_c[:], brun[:])
                bi16 = small.tile([128, H], bf16, name="bi16", tag="bi16")
                nc.vector.tensor_mul(bi16[:], ainv[:], binvr[:])
                if has_mask:
                    nc.vector.tensor_mul(
                        b16[:], b16[:],
                        mkc_sb[:, ch:ch + 1].to_broadcast([128, H]))
                    nc.vector.tensor_mul(
                        bi16[:], bi16[:],
                        mkc_sb[:, ch:ch + 1].to_broadcast([128, H]))

                gl_sb = small.tile([1, H], f32, name="gl_sb", tag="gl_sb")
                nc.vector.tensor_scalar_max(gl_sb[:], glp[:], G_CLAMP)
                egl = small.tile([1, 2, H], f32, name="egl", tag="egl")
                nc.scalar.activation(egl[:, 0, :], gl_sb[:], AF.Exp)
                nc.scalar.activation(egl[:, 1, :], gl_sb[:], AF.Exp, scale=-1.0)
                bca = sp.tile([128, 2 * H], f32, name="bca", tag="sps")
                nc.tensor.matmul(bca[:], ones_row[:],
                                 egl.rearrange("o t h -> o (t h)"),
                                 start=True, stop=True)
                nc.vector.tensor_mul(brun[:], brun[:], bca[:, 0:H])
                nc.vector.tensor_mul(binvr[:], binvr[:], bca[:, H:2 * H])
                nc.vector.tensor_scalar_min(binvr[:], binvr[:], 1e26)

                # ---- q/k: project, rope, phi, decay-scale, transpose
                qt_c = qtp.tile([64, H, 128], bf16, name=f"qt{ch}", tag="qt")
                kt_c = work.tile([64, H, 128], bf16, name="kt_c", tag="kt_c")
                ktm = work.tile([128, H, D], bf16, name="ktm", tag="ktm")

                for which in range(2):  # 0 = q, 1 = k
                    prj = pp.tile([128, C], f32, name="prj", tag="prj")
                    off = which * C
                    for kt in range(KT):
                        for nh in range(2):
                            nc.tensor.matmul(
                                prj[:, bass.ts(nh, 512)],
                                xtb_sb[:, kt, tsl],
                                wb_sb[:, kt, off + nh * 512:off + (nh + 1) * 512],
                                start=(kt == 0), stop=(kt == KT - 1))
                    # copy psum -> sbuf bf16, then rope there
                    psb = work.tile([128, H, D], bf16, name="psb", tag="psb")
                    nc.scalar.copy(psb.rearrange("p h d -> p (h d)"), prj[:])
                    rr = work.tile([128, H, D], bf16, name="rr", tag="rr")
                    tmp = work.tile([128, H, D], bf16, name="tmp", tag="tmp")
                    cs = rope_sb[:, ch, 0:64]
                    sna = rope_sb[:, ch, 64:96]
                    snb = rope_sb[:, ch, 96:128]
                    nc.vector.tensor_mul(
                        rr[:], psb[:], cs[:, None, :].to_broadcast([128, H, D]))
                    nc.vector.tensor_mul(
                        tmp[:, :, 0:32], psb[:, :, 32:64],
                        sna[:, None, :].to_broadcast([128, H, 32]))
                    nc.gpsimd.tensor_mul(
                        tmp[:, :, 32:64], psb[:, :, 0:32],
                        snb[:, None, :].to_broadcast([128, H, 32]))
                    nc.vector.tensor_add(rr[:], rr[:], tmp[:])
                    # phi(x) = exp(min(x,0)) + max(x,0), then decay scale
                    mn = work.tile([128, H, D], bf16, name="mn", tag="mn")
                    nc.vector.tensor_scalar_min(mn[:], rr[:], 0.0)
                    nc.scalar.activation(mn[:], mn[:], AF.Exp)
                    nc.gpsimd.tensor_scalar_max(rr[:], rr[:], 0.0)
                    nc.vector.tensor_add(rr[:], rr[:], mn[:])
                    scale = b16 if which == 0 else bi16
                    nc.vector.tensor_mul(
                        rr[:], rr[:], scale[:, :, None].to_broadcast([128, H, D]))
                    if which == 1:
                        nc.vector.tensor_copy(ktm[:], rr[:])
                    # transpose to D-major, head-major [64, H, 128]
                    dst = qt_c if which == 0 else kt_c
                    for tg in range(4):
                        tps = tp.tile([64, 4, 128], bf16, name="tps", tag="tps")
                        for j in range(4):
                            h = tg * 4 + j
                            nc.tensor.matmul(
                                tps[:, j, :],
                                rr.rearrange("p h d -> p (h d)")[:, bass.ts(h, 64)],
                                eye_sb[:], is_transpose=True,
                                start=(j == 0), stop=(j == 3))
                        if tg % 2 == 0:
                            nc.vector.tensor_copy(dst[:, tg * 4:(tg + 1) * 4, :],
                                                  tps[:])
                        else:
                            nc.scalar.copy(dst[:, tg * 4:(tg + 1) * 4, :], tps[:])

                # ---- v: project + ones column (+ mask)
                vext = work.tile([128, H, SE], bf16, name="vext", tag="vext")
                vps = pp.tile([128, C], f32, name="vps", tag="prj")
                for kt in range(KT):
                    for nh in range(2):
                        nc.tensor.matmul(
                            vps[:, bass.ts(nh, 512)],
                            xtb_sb[:, kt, tsl],
                            wb_sb[:, kt, 2 * C + nh * 512:2 * C + (nh + 1) * 512],
                            start=(kt == 0), stop=(kt == KT - 1))
                if has_mask:
                    nc.vector.tensor_mul(
                        vext[:, :, 0:D], vps.rearrange("p (h d) -> p h d", h=H),
                        mkc_sb[:, ch:ch + 1, None].to_broadcast([128, H, D]))
                else:
                    nc.scalar.copy(vext[:, :, 0:D],
                                   vps.rearrange("p (h d) -> p h d", h=H))
                nc.vector.tensor_copy(vext[:, :, D],
                                      nc.const_aps.aps[(bf16, 1.0)]
                                      .to_broadcast([128, H]))

                # ---- At (causal-masked) and intra-chunk numerators
                ni_c = nip.tile([128, H, SE], f32, name=f"ni{ch}", tag="ni")
                for g in range(4):
                    atp = ap_.tile([128, 512], f32, name="atp", tag="atp")
                    for j in range(4):
                        h = g * 4 + j
                        nc.tensor.matmul(atp[:, bass.ts(j, 128)],
                                         kt_c[:, h, :], qt_c[:, h, :],
                                         start=(j == 0), stop=(j == 3))
                    atm = work.tile([128, 4, 128], bf16, name="atm", tag="atm")
                    nc.vector.tensor_mul(
                        atm[:], atp.rearrange("p (j t) -> p j t", j=4),
                        triu_sb[:, None, :].to_broadcast([128, 4, 128]))
                    nps = ap_.tile([128, 4, SE], f32, name="nps", tag="nps")
                    for j in range(4):
                        h = g * 4 + j
                        nc.tensor.matmul(nps[:, j, :], atm[:, j, :],
                                         vext[:, h, :],
                                         start=(j == 0), stop=(j == 3))
                    nc.scalar.copy(ni_c[:, g * 4:(g + 1) * 4, :], nps[:])

                # ---- ΔU and U snapshot (head-major [64, H, SE], fp32)
                u_c = usp.tile([64, H, SE], f32, name=f"u{ch}", tag="u")
                for g in range(4):
                    ups = up.tile([64, 4, SE], f32, name="ups", tag="ups")
                    for j in range(4):
                        h = g * 4 + j
                        nc.tensor.matmul(ups[:, j, :], ktm[:, h, :],
                                         vext[:, h, :],
                                         start=(j == 0), stop=(j == 3))
                    dst = u_c[:, g * 4:(g + 1) * 4, :]
                    if ch == 0:
                        nc.vector.tensor_copy(dst, ups[:])
                    else:
                        nc.vector.tensor_add(
                            dst, ups[:], u_tiles[ch - 1][:, g * 4:(g + 1) * 4, :])
                qt_tiles.append(qt_c)
                ni_tiles.append(ni_c)
                u_tiles.append(u_c)

        # ----------------------------------------------- phase 1b: exchange
        seff = keep.tile([64, H, SE], f32, name="seff")
        nc.vector.tensor_mul(
            seff[:], u_tiles[-1][:],
            brun[0:64, :, None].to_broadcast([64, H, SE]))

        cc_in = dram.tile([64, H * SE + H], f32, name="cc_in")
        cc_out = dram.tile([256, H * SE + H], f32, name="cc_out")
        nc.sync.dma_start(cc_in[:, 0:H * SE], seff.rearrange("p h e -> p (h e)"))
        nc.sync.dma_start(cc_in[:, H * SE:], brun[0:64, :])
        nc.gpsimd.collective_compute(
            "AllGather", ALU.bypass,
            replica_groups=[[0, 1, 2, 3], [4, 5, 6, 7]],
            ins=[cc_in.opt()], outs=[cc_out.opt()])

        sstart = keep.tile([64, H, SE], f32, name="sstart")
        nc.vector.tensor_copy(
            sstart.rearrange("p h e -> p (h e)"),
            nc.const_aps.aps[(f32, 0.0)][0:64].to_broadcast([64, H * SE]))
        for q in range(4):
            peer = keep.tile([64, H * SE + H], f32, name=f"peer{q}")
            nc.sync.dma_start(peer[:], cc_out[q * 64:(q + 1) * 64, :])
            aq = keep.tile([64, H], f32, name=f"aq{q}")
            # M = A_q*sel + (1-sel) ; S = S*M + sel*Seff_q
            nc.vector.tensor_mul(
                aq[:], peer[:, H * SE:],
                sel_sb[0:64, q:q + 1].to_broadcast([64, H]))
            nc.vector.tensor_add(
                aq[:], aq[:], isel_sb[0:64, q:q + 1].to_broadcast([64, H]))
            nc.vector.tensor_mul(
                sstart[:], sstart[:], aq[:, :, None].to_broadcast([64, H, SE]))
            psel = keep.tile([64, H, SE], f32, name=f"psel{q}")
            nc.vector.tensor_mul(
                psel.rearrange("p h e -> p (h e)"), peer[:, 0:H * SE],
                sel_sb[0:64, q:q + 1].to_broadcast([64, H * SE]))
            nc.vector.tensor_add(sstart[:], sstart[:], psel[:])

        # ---------------------------------------------------------- phase 2
        with contextlib.ExitStack() as ph2:
            wp2 = ph2.enter_context(tc.tile_pool(name="wp2", bufs=1))
            wk2 = ph2.enter_context(tc.tile_pool(name="wk2", bufs=2))
            ip = ph2.enter_context(tc.tile_pool(name="ip", bufs=2, space="PSUM"))
            tp2 = ph2.enter_context(tc.tile_pool(name="tp2", bufs=2, space="PSUM"))
            op = ph2.enter_context(tc.tile_pool(name="op", bufs=2, space="PSUM"))

            wo_sb = wp2.tile([128, KT, C], bf16, name="wo_sb")
            for kt in range(KT):
                nc.sync.dma_start(wo_sb[:, kt, :], P["wo"][:, kt, :])
            bo_sb = wp2.tile([1, C], f32, name="bo_sb")
            nc.sync.dma_start(bo_sb[:], P["bo2"][:])
            if has_ln:
                lnw_sb = wp2.tile([128, C], f32, name="lnw_sb")
                lnb_sb = wp2.tile([128, C], f32, name="lnb_sb")
                nc.sync.dma_start(lnw_sb[:], P["lnw"][:])
                nc.sync.dma_start(lnb_sb[:], P["lnb"][:])

            for ch in range(NCH):
                if ch == 0:
                    sacc = sstart
                else:
                    sacc = wk2.tile([64, H, SE], f32, name="sacc", tag="sacc")
                    nc.vector.tensor_add(sacc[:], sstart[:], u_tiles[ch - 1][:])
                sacb = wk2.tile([64, H, SE], bf16, name="sacb", tag="sacb")
                nc.vector.tensor_copy(sacb[:], sacc[:])

                attn = wk2.tile([128, H, D], f32, name="attn", tag="attn")
                den = wk2.tile([128, H], f32, name="den", tag="den")
                tnum = wk2.tile([128, H, SE], f32, name="tnum", tag="tnum")
                for g in range(4):
                    ips = ip.tile([128, 4, SE], f32, name="ips", tag="ips")
                    for j in range(4):
                        h = g * 4 + j
                        nc.tensor.matmul(ips[:, j, :], qt_tiles[ch][:, h, :],
                                         sacb[:, h, :],
                                         start=(j == 0), stop=(j == 3))
                    sl = slice(g * 4, (g + 1) * 4)
                    nc.vector.tensor_add(tnum[:, sl, :], ips[:],
                                         ni_tiles[ch][:, sl, :])
                    nc.vector.tensor_copy(den[:, sl], tnum[:, sl, D])
                nc.vector.tensor_scalar_max(den[:], den[:], EPS)
                nc.vector.reciprocal(den[:], den[:])
                nc.vector.tensor_mul(attn[:], tnum[:, :, 0:D],
                                     den[:, :, None].to_broadcast([128, H, D]))

                # transpose attn -> C-major, then o-proj (+bias) + residual
                at_sb = wk2.tile([128, KT, 128], bf16, name="at_sb", tag="at_sb")
                for tg in range(2):
                    tps = tp2.tile([128, 512], f32, name="tps2", tag="tps2")
                    for j in range(4):
                        nc.tensor.matmul(
                            tps[:, bass.ts(j, 128)],
                            attn.rearrange("p h d -> p (h d)")[
                                :, bass.ts(tg * 4 + j, 128)],
                            eyef_sb[:], is_transpose=True,
                            start=(j == 0), stop=(j == 3))
                    if tg == 0:
                        nc.vector.tensor_copy(
                            at_sb.rearrange("p k t -> p (k t)")[:, 0:512], tps[:])
                    else:
                        nc.scalar.copy(
                            at_sb.rearrange("p k t -> p (k t)")[:, 512:1024],
                            tps[:])

                ops = op.tile([128, C], f32, name="ops", tag="ops")
                for nh in range(2):
                    nsl = bass.ts(nh, 512)
                    for kt in range(KT):
                        nc.tensor.matmul(ops[:, nsl], at_sb[:, kt, :],
                                         wo_sb[:, kt, nsl],
                                         start=(kt == 0), stop=False)
                    nc.tensor.matmul(ops[:, nsl], ones_row[:],
                                     bo_sb[:, nsl], start=False, stop=True)

                xr = wk2.tile([128, C], f32, name="xr", tag="xr")
                nc.sync.dma_start(xr[:], P["xrows"][bass.ts(ch, 128), :])
                ysb = wk2.tile([128, C], f32, name="ysb", tag="ysb")
                nc.vector.tensor_add(ysb[:], ops[:], xr[:])

                # LayerNorm
                mus = wk2.tile([128, 4], f32, name="mus", tag="mus")
                scr = wk2.tile([128, C], f32, name="scr", tag="scr")
                nc.scalar.activation(scr[:], ysb[:], AF.Identity,
                                     accum_out=mus[:, 0:1])
                nc.vector.tensor_scalar_mul(mus[:, 1:2], mus[:, 0:1], -1.0 / C)
                nc.scalar.activation(scr[:], ysb[:], AF.Square,
                                     bias=mus[:, 1:2], accum_out=mus[:, 2:3])
                nc.vector.tensor_scalar(mus[:, 2:3], mus[:, 2:3], 1.0 / C,
                                        LN_EPS, ALU.mult, ALU.add)
                nc.scalar.activation(mus[:, 2:3], mus[:, 2:3], AF.Sqrt)
                nc.vector.reciprocal(mus[:, 2:3], mus[:, 2:3])
                nc.vector.tensor_mul(mus[:, 3:4], mus[:, 1:2], mus[:, 2:3])
                yln = wk2.tile([128, C], f32, name="yln", tag="yln")
                nc.scalar.activation(yln[:], ysb[:], AF.Identity,
                                     scale=mus[:, 2:3], bias=mus[:, 3:4])
                if has_ln:
                    nc.vector.tensor_mul(yln[:], yln[:], lnw_sb[:])
                    nc.vector.tensor_add(yln[:], yln[:], lnb_sb[:])
                nc.sync.dma_start(P["y"][bass.ts(ch, 128), :], yln[:])

    if SPLIT_WAITS:
        _split_multi_waits(nc)
    return nc


# ---------------------------------------------------------------- host side
def _rope_tables():
    half = D // 2
    inv = 1.0 / (ROPE_BASE ** (np.arange(half, dtype=np.float64) / half))
    t = np.arange(T, dtype=np.float64)
    fr = t[:, None] * inv[None, :]
    cos, sin = np.cos(fr), np.sin(fr)
    out = np.zeros((T, 128), np.float32)
    out[:, 0:32] = cos
    out[:, 32:64] = cos
    out[:, 64:96] = -sin
    out[:, 96:128] = sin
    return out


def _ktile(w, dtype=np.float32):  # [C, N] -> [128, KT, N]
    return np.ascontiguousarray(
        w.reshape(KT, 128, w.shape[1]).transpose(1, 0, 2)).astype(dtype)


_cache = {}
RUN_KW = {}      # extra kwargs for run_bass_kernel_spmd (test harness profiling)
LAST = None      # last BassKernelResults (test harness reads exec_time_ns)


def kernel(x, mask, Wq, Wk, Wv, Wg, bg, Wo, bo, ln_w, ln_b):
    bfl = ml_dtypes.bfloat16
    x = np.asarray(x, np.float32)
    mask = np.asarray(mask)
    has_mask = not np.all(mask == 1)
    has_ln = not (np.all(np.asarray(ln_w) == 1) and np.all(np.asarray(ln_b) == 0))

    key = (has_mask, has_ln)
    if key not in _cache:
        _cache[key] = build(has_mask, has_ln)
    nc = _cache[key]

    wqkv = _ktile(np.concatenate(
        [np.asarray(Wq).T, np.asarray(Wk).T, np.asarray(Wv).T], axis=1), bfl)
    wg = _ktile(np.ascontiguousarray(np.asarray(Wg, np.float32).T))
    wo_t = _ktile(np.ascontiguousarray(np.asarray(Wo).T), bfl)
    ropec_full = _rope_tables()
    triu = np.triu(np.ones((128, 128), np.float32))
    eye = np.eye(128)
    onesrow = np.ones((1, 128), np.float32)
    bgt = np.tile(np.asarray(bg, np.float32), (128, 1))
    bo2 = np.asarray(bo, np.float32)[None, :]

    in_maps = []
    for c in range(NCORE):
        b, r = c // 4, c % 4
        rows = slice(r * QT, (r + 1) * QT)
        xq = np.ascontiguousarray(x[b, rows].T)   # [C, QT]
        m = {
            "xT": _ktile(xq),
            "xTb": _ktile(xq, bfl),
            "wqkv": wqkv,
            "wg": wg,
            "wo": wo_t,
            "xrows": np.ascontiguousarray(x[b, rows]),
            "bo2": bo2,
            "ropec": np.ascontiguousarray(
                ropec_full[rows].reshape(NCH, 128, 128).transpose(1, 0, 2)),
            "triu": triu,
            "eye": eye.astype(bfl),
            "eyef": eye.astype(np.float32),
            "onesrow": onesrow,
            "bgt": bgt,
        }
        sel = np.zeros((128, 4), np.float32)
        sel[:, 0:r] = 1.0
        m["sel"] = sel
        m["isel"] = 1.0 - sel
        if has_mask:
            mk = np.asarray(mask[b, rows], np.float32)
            m["mkc"] = np.ascontiguousarray(mk.reshape(NCH, 128).T)
            m["mki"] = 1.0 - m["mkc"]
        if has_ln:
            m["lnw"] = np.tile(np.asarray(ln_w, np.float32), (128, 1))
            m["lnb"] = np.tile(np.asarray(ln_b, np.float32), (128, 1))
        in_maps.append(m)

    res = run_bass_kernel_spmd(nc, in_maps, list(range(NCORE)), **RUN_KW)
    globals()["LAST"] = res
    out = np.empty((B, T, C), np.float32)
    for c in range(NCORE):
        b, r = c // 4, c % 4
        out[b, r * QT:(r + 1) * QT, :] = res.results[c]["y"]
    return out


# revision 14
# speedup vs baseline: 1.4772x; 1.4772x over previous
"""DeltaNet-style gated linear attention forward on 8 Trainium2 NeuronCores.

Sharding: core c = (batch b = c//4, sequence quarter r = c%4). Each core
projects q/k/v/gate for its 512 rows (all 16 heads), runs chunked linear
attention in quarter-local unscaled coordinates, exchanges per-quarter state
summaries via a small AllGather, then applies the inter-quarter state, output
projection, residual and LayerNorm for its own rows.

Math (per batch, head), matching the reference scan: with
b_i = prod_{j<=i} f_j (cumprod from quarter start), q~_i = phi_i * b_i,
k~_j = phi_j / b_j, the output row i inside a quarter is
  out_i = q~_i (S_start + U_i) / max(q~_i . (m_start + mU_i), eps)
where U_i = sum_{j<=i, same quarter} k~_j v_j^T accumulates unscaled and
(S_start, m_start) is the true state entering the quarter, combined from the
peer quarters' summaries (A_q, A_q * U_q) after an AllGather.

Precision: projections and attention matmuls run in bf16 (inputs) with fp32
PSUM accumulation; the gate/decay chain, state accumulation, normalization
and LayerNorm stay fp32.
"""

import numpy as np
import ml_dtypes

import bass_rust
import concourse.bass as bass
import concourse.mybir as mybir
import concourse.tile as tile
from concourse.bass_utils import run_bass_kernel_spmd

dt = mybir.dt
AF = mybir.ActivationFunctionType
ALU = mybir.AluOpType

B, T, C, H, D = 2, 2048, 1024, 16, 64
NCORE = 8
QT = T // 4          # rows per core
L = 128              # chunk length
NCH = QT // L        # chunks per core
KT = C // 128        # contraction tiles
SE = D + 1           # state row width (S | m)
ROPE_BASE = 10000.0
EPS = 1e-6
LN_EPS = 1e-5
G_CLAMP = -30.0      # per-chunk cumsum floor (defensive; inert for real data)
SPLIT_WAITS = True   # walrus here takes <=1 sem wait per instruction


# ---------------------------------------------------------------- walrus shim
def _split_multi_waits(nc):
    ctr = 0
    for fn in nc.m.functions:
        for bb in fn.blocks:
            out = []
            for ins in bb.instructions:
                si = ins.sync_info
                if si is not None and si.on_wait and len(si.on_wait) > 1:
                    waits = list(si.on_wait)
                    for w in waits[:-1]:
                        ctr += 1
                        nop = mybir.InstNoOp(name=f"WS-{ctr}", ins=[], outs=[])
                        nop.engine = ins.engine
                        nop.sync_info = bass_rust.SyncInfo(on_wait=[w], on_update=[])
                        nop.debug = ins.debug
                        out.append(nop)
                    si.on_wait = [waits[-1]]
                out.append(ins)
            bb.instructions[:] = out
    return ctr


def _register_const(nc, value, dtype=dt.float32):
    t = nc.alloc_sbuf_tensor(f"uconst-{dtype.name}-{value}", [128, 1], dtype)
    nc.gpsimd.memset(t.ap(), value)
    nc.const_aps.aps[(dtype, value)] = t.ap()


# ------------------------------------------------------------------- builder
def _enable_ldw_opt():
    try:
        from concourse.compiler_utils import get_compiler_flags, set_compiler_flags
        flags = get_compiler_flags()
        new = [f.replace("--enable-ldw-opt=false", "--enable-ldw-opt=true")
               for f in flags]
        if new != flags:
            set_compiler_flags(new)
    except Exception:
        pass


def build(has_mask=False, has_ln=False):
    _enable_ldw_opt()
    nc = bass.Bass(target_bir_lowering=False, debug=False)
    _register_const(nc, float(LN_EPS))
    nc.all_engine_barrier()

    f32 = dt.float32
    bf16 = dt.bfloat16
    P = {}

    def param(name, shape, dtype=f32, out=False):
        P[name] = nc.declare_dram_parameter(name, list(shape), dtype, isOutput=out)
        return P[name]

    param("xT", (128, KT, QT))                  # x rows^T (fp32, gate path)
    param("xTb", (128, KT, QT), bf16)           # x rows^T (bf16, qkv path)
    param("wqkv", (128, KT, 3 * C), bf16)       # [Wq.T|Wk.T|Wv.T] k-tiled
    param("wg", (128, KT, H))                   # Wg.T k-tiled (fp32)
    param("wo", (128, KT, C), bf16)             # Wo.T k-tiled
    param("xrows", (QT, C))                     # residual rows
    param("bo2", (1, C))                        # output bias row
    param("ropec", (128, NCH, 128))             # [cos|cos|-sin|+sin] per chunk
    param("triu", (128, 128))                   # j<=i ones (cumsum + causal)
    param("eye", (128, 128), bf16)              # PE transpose identity (bf16)
    param("eyef", (128, 128))                   # PE transpose identity (fp32)
    param("onesrow", (1, 128))
    param("bgt", (128, H))                      # gate bias, broadcast
    param("sel", (128, 4))                      # quarter-combine select (q < r)
    param("isel", (128, 4))                     # 1 - sel
    if has_mask:
        param("mkc", (128, NCH))
        param("mki", (128, NCH))
    if has_ln:
        param("lnw", (128, C))
        param("lnb", (128, C))
    param("y", (QT, C), out=True)

    one_ap = nc.const_aps.aps[(f32, 1.0)]

    import contextlib
    with tile.TileContext(nc) as tc, contextlib.ExitStack() as outer:
        keep = outer.enter_context(tc.tile_pool(name="keep", bufs=1))
        qtp = outer.enter_context(tc.tile_pool(name="qtp", bufs=NCH))
        nip = outer.enter_context(tc.tile_pool(name="nip", bufs=NCH))
        usp = outer.enter_context(tc.tile_pool(name="usp", bufs=NCH))
        dram = outer.enter_context(tc.tile_pool(name="dram", bufs=1, space="DRAM"))

        # constants
        triu_sb = keep.tile([128, 128], f32, name="triu_sb")
        eye_sb = keep.tile([128, 128], bf16, name="eye_sb")
        eyef_sb = keep.tile([128, 128], f32, name="eyef_sb")
        ones_row = keep.tile([1, 128], f32, name="ones_row")
        bgt_sb = keep.tile([128, H], f32, name="bgt_sb")
        rope_sb = keep.tile([128, NCH, 128], f32, name="rope_sb")
        sel_sb = keep.tile([128, 4], f32, name="sel_sb")
        isel_sb = keep.tile([128, 4], f32, name="isel_sb")
        for t_, p_ in ((triu_sb, "triu"), (eye_sb, "eye"), (eyef_sb, "eyef"),
                       (ones_row, "onesrow"), (bgt_sb, "bgt"),
                       (rope_sb, "ropec"), (sel_sb, "sel"), (isel_sb, "isel")):
            nc.sync.dma_start(t_[:], P[p_][:])
        if has_mask:
            mkc_sb = keep.tile([128, NCH], f32, name="mkc_sb")
            mki_sb = keep.tile([128, NCH], f32, name="mki_sb")
            nc.sync.dma_start(mkc_sb[:], P["mkc"][:])
            nc.sync.dma_start(mki_sb[:], P["mki"][:])

        qt_tiles, ni_tiles, u_tiles = [], [], []
        brun = keep.tile([128, H], f32, name="brun")
        binvr = keep.tile([128, H], f32, name="binvr")
        nc.vector.tensor_copy(brun[:], one_ap.to_broadcast([128, H]))
        nc.vector.tensor_copy(binvr[:], one_ap.to_broadcast([128, H]))

        # ---------------------------------------------------------- phase 1
        with contextlib.ExitStack() as ph1:
            wp = ph1.enter_context(tc.tile_pool(name="wp", bufs=1))
            work = ph1.enter_context(tc.tile_pool(name="work", bufs=2))
            small = ph1.enter_context(tc.tile_pool(name="small", bufs=2))
            pp = ph1.enter_context(tc.tile_pool(name="pp", bufs=1, space="PSUM"))
            sp = ph1.enter_context(tc.tile_pool(name="sp", bufs=1, space="PSUM"))
            tp = ph1.enter_context(tc.tile_pool(name="tp", bufs=1, space="PSUM"))
            ap_ = ph1.enter_context(tc.tile_pool(name="ap", bufs=1, space="PSUM"))
            up = ph1.enter_context(tc.tile_pool(name="up", bufs=1, space="PSUM"))

            wb_sb = wp.tile([128, KT, 3 * C], bf16, name="wb_sb")
            wg_sb = wp.tile([128, KT, H], f32, name="wg_sb")
            xt_sb = wp.tile([128, KT, QT], f32, name="xt_sb")
            xtb_sb = wp.tile([128, KT, QT], bf16, name="xtb_sb")
            for kt in range(KT):
                nc.sync.dma_start(wb_sb[:, kt, :], P["wqkv"][:, kt, :])
                nc.sync.dma_start(xt_sb[:, kt, :], P["xT"][:, kt, :])
                nc.sync.dma_start(xtb_sb[:, kt, :], P["xTb"][:, kt, :])
            nc.sync.dma_start(wg_sb[:], P["wg"][:])

            for ch in range(NCH):
                tsl = bass.ts(ch, L)

                # ---- gate: f = clip(sigmoid(z + bg)); logf; cumsums
                fz = sp.tile([128, H], f32, name="fz", tag="sps")
                for kt in range(KT):
                    nc.tensor.matmul(fz[:], xt_sb[:, kt, tsl],
                                     wg_sb[:, kt, :],
                                     start=(kt == 0), stop=(kt == KT - 1))
                logf = small.tile([128, H], f32, name="logf", tag="logf")
                nc.vector.tensor_add(logf[:], fz[:], bgt_sb[:])
                nc.scalar.activation(logf[:], logf[:], AF.Sigmoid)
                nc.vector.tensor_scalar(logf[:], logf[:], 0.999, 0.01,
                                        ALU.min, ALU.max)
                if has_mask:
                    nc.vector.tensor_mul(
                        logf[:], logf[:],
                        mkc_sb[:, ch:ch + 1].to_broadcast([128, H]))
                    nc.vector.tensor_add(
                        logf[:], logf[:],
                        mki_sb[:, ch:ch + 1].to_broadcast([128, H]))
                nc.scalar.activation(logf[:], logf[:], AF.Ln)

                gps = sp.tile([128, H], f32, name="gps", tag="sps")
                nc.tensor.matmul(gps[:], triu_sb[:], logf[:],
                                 start=True, stop=True)
                glp = sp.tile([1, H], f32, name="glp", tag="sps")
                nc.tensor.matmul(glp[:], one_ap[:, 0:1], logf[:],
                                 start=True, stop=True)

                a_c = small.tile([128, H], f32, name="a_c", tag="a_c")
                nc.vector.tensor_scalar_max(a_c[:], gps[:], G_CLAMP)
                ainv = small.tile([128, H], f32, name="ainv", tag="ainv")
                nc.scalar.activation(ainv[:], a_c[:], AF.Exp, scale=-1.0)
                nc.scalar.activation(a_c[:], a_c[:], AF.Exp)

                b16 = small.tile([128, H], bf16, name="b16", tag="b16")
                nc.vector.tensor_mul(b16[:], a_c[:], brun[:])
                bi16 = small.tile([128, H], bf16, name="bi16", tag="bi16")
                nc.vector.tensor_mul(bi16[:], ainv[:], binvr[:])
                if has_mask:
                    nc.vector.tensor_mul(
                        b16[:], b16[:],
                        mkc_sb[:, ch:ch + 1].to_broadcast([128, H]))
                    nc.vector.tensor_mul(
                        bi16[:], bi16[:],
                        mkc_sb[:, ch:ch + 1].to_broadcast([128, H]))

                gl_sb = small.tile([1, H], f32, name="gl_sb", tag="gl_sb")
                nc.vector.tensor_scalar_max(gl_sb[:], glp[:], G_CLAMP)
                egl = small.tile([1, 2, H], f32, name="egl", tag="egl")
                nc.scalar.activation(egl[:, 0, :], gl_sb[:], AF.Exp)
                nc.scalar.activation(egl[:, 1, :], gl_sb[:], AF.Exp, scale=-1.0)
                bca = sp.tile([128, 2 * H], f32, name="bca", tag="sps")
                nc.tensor.matmul(bca[:], ones_row[:],
                                 egl.rearrange("o t h -> o (t h)"),
                                 start=True, stop=True)
                nc.vector.tensor_mul(brun[:], brun[:], bca[:, 0:H])
                nc.vector.tensor_mul(binvr[:], binvr[:], bca[:, H:2 * H])
                nc.vector.tensor_scalar_min(binvr[:], binvr[:], 1e26)

                # ---- q/k: project, rope, phi, decay-scale, transpose
                qt_c = qtp.tile([64, H, 128], bf16, name=f"qt{ch}", tag="qt")
                kt_c = work.tile([64, H, 128], bf16, name="kt_c", tag="kt_c")
                ktm = work.tile([128, H, D], bf16, name="ktm", tag="ktm")

                qkps = pp.tile([128, 2 * C], f32, name="qkps", tag="prj")
                for kt in range(KT):
                    for nh in range(4):  # q0 q1 k0 k1 share this kt's lhsT
                        nc.tensor.matmul(
                            qkps[:, bass.ts(nh, 512)],
                            xtb_sb[:, kt, tsl],
                            wb_sb[:, kt, nh * 512:(nh + 1) * 512],
                            start=(kt == 0), stop=(kt == KT - 1))
                for which in range(2):  # 0 = q, 1 = k
                    prj = qkps[:, which * C:(which + 1) * C]
                    # copy psum -> sbuf bf16, then rope there
                    psb = work.tile([128, H, D], bf16, name="psb", tag="psb")
                    nc.scalar.copy(psb.rearrange("p h d -> p (h d)"), prj[:])
                    rr = work.tile([128, H, D], bf16, name="rr", tag="rr")
                    tmp = work.tile([128, H, D], bf16, name="tmp", tag="tmp")
                    cs = rope_sb[:, ch, 0:64]
                    sna = rope_sb[:, ch, 64:96]
                    snb = rope_sb[:, ch, 96:128]
                    nc.vector.tensor_mul(
                        rr[:], psb[:], cs[:, None, :].to_broadcast([128, H, D]))
                    nc.vector.tensor_mul(
                        tmp[:, :, 0:32], psb[:, :, 32:64],
                        sna[:, None, :].to_broadcast([128, H, 32]))
                    nc.vector.tensor_mul(
                        tmp[:, :, 32:64], psb[:, :, 0:32],
                        snb[:, None, :].to_broadcast([128, H, 32]))
                    rrf = rr.rearrange("p h d -> p (h d)")
                    nc.vector.tensor_add(rrf, rrf,
                                         tmp.rearrange("p h d -> p (h d)"))
                    # phi(x) = exp(min(x,0)) + max(x,0), then decay scale
                    mn = work.tile([128, H, D], bf16, name="mn", tag="mn")
                    mnf = mn.rearrange("p h d -> p (h d)")
                    nc.vector.tensor_scalar_min(mnf, rrf, 0.0)
                    nc.scalar.activation(mnf, mnf, AF.Exp)
                    nc.scalar.activation(rrf, rrf, AF.Relu)
                    nc.vector.tensor_add(rrf, rrf, mnf)
                    scale = b16 if which == 0 else bi16
                    nc.vector.tensor_mul(
                        rr[:], rr[:], scale[:, :, None].to_broadcast([128, H, D]))
                    if which == 1:
                        nc.vector.tensor_copy(ktm.rearrange("p h d -> p (h d)"),
                                              rrf)
                    # transpose to D-major, head-major [64, H, 128]
                    dst = qt_c if which == 0 else kt_c
                    for tg in range(4):
                        tps = tp.tile([64, 4, 128], bf16, name="tps", tag="tps")
                        for j in range(4):
                            h = tg * 4 + j
                            nc.tensor.matmul(
                                tps[:, j, :],
                                rr.rearrange("p h d -> p (h d)")[:, bass.ts(h, 64)],
                                eye_sb[:], is_transpose=True,
                                start=(j == 0), stop=(j == 3))
                        if tg % 2 == 0:
                            nc.vector.tensor_copy(dst[:, tg * 4:(tg + 1) * 4, :],
                                                  tps[:])
                        else:
                            nc.scalar.copy(dst[:, tg * 4:(tg + 1) * 4, :], tps[:])

                # ---- v: project + ones column (+ mask)
                vext = work.tile([128, H, SE], bf16, name="vext", tag="vext")
                vps = pp.tile([128, C], f32, name="vps", tag="prj")
                for kt in range(KT):
                    for nh in range(2):
                        nc.tensor.matmul(
                            vps[:, bass.ts(nh, 512)],
                            xtb_sb[:, kt, tsl],
                            wb_sb[:, kt, 2 * C + nh * 512:2 * C + (nh + 1) * 512],
                            start=(kt == 0), stop=(kt == KT - 1))
                if has_mask:
                    nc.vector.tensor_mul(
                        vext[:, :, 0:D], vps.rearrange("p (h d) -> p h d", h=H),
                        mkc_sb[:, ch:ch + 1, None].to_broadcast([128, H, D]))
                else:
                    nc.scalar.copy(vext[:, :, 0:D],
                                   vps.rearrange("p (h d) -> p h d", h=H))
                nc.vector.tensor_copy(vext[:, :, D],
                                      nc.const_aps.aps[(bf16, 1.0)]
                                      .to_broadcast([128, H]))

                # ---- At (causal-masked) and intra-chunk numerators
                ni_c = nip.tile([128, H, SE], f32, name=f"ni{ch}", tag="ni")
                for g in range(4):
                    atp = ap_.tile([128, 512], f32, name="atp", tag="atp")
                    for j in range(4):
                        h = g * 4 + j
                        nc.tensor.matmul(atp[:, bass.ts(j, 128)],
                                         kt_c[:, h, :], qt_c[:, h, :],
                                         start=(j == 0), stop=(j == 3))
                    atm = work.tile([128, 4, 128], bf16, name="atm", tag="atm")
                    nc.vector.tensor_mul(
                        atm[:], atp.rearrange("p (j t) -> p j t", j=4),
                        triu_sb[:, None, :].to_broadcast([128, 4, 128]))
                    nps = ap_.tile([128, 4, SE], f32, name="nps", tag="atp")
                    for j in range(4):
                        h = g * 4 + j
                        nc.tensor.matmul(nps[:, j, :], atm[:, j, :],
                                         vext[:, h, :],
                                         start=(j == 0), stop=(j == 3))
                    nc.scalar.copy(ni_c[:, g * 4:(g + 1) * 4, :], nps[:])

                # ---- ΔU and U snapshot (head-major [64, H, SE], fp32)
                u_c = usp.tile([64, H, SE], f32, name=f"u{ch}", tag="u")
                for g in range(4):
                    ups = up.tile([64, 4, SE], f32, name="ups", tag="ups")
                    for j in range(4):
                        h = g * 4 + j
                        nc.tensor.matmul(ups[:, j, :], ktm[:, h, :],
                                         vext[:, h, :],
                                         start=(j == 0), stop=(j == 3))
                    dst = u_c[:, g * 4:(g + 1) * 4, :]
                    if ch == 0:
                        nc.vector.tensor_copy(dst, ups[:])
                    else:
                        nc.vector.tensor_add(
                            dst, ups[:], u_tiles[ch - 1][:, g * 4:(g + 1) * 4, :])
                qt_tiles.append(qt_c)
                ni_tiles.append(ni_c)
                u_tiles.append(u_c)

        # ----------------------------------------------- phase 1b: exchange
        seff = keep.tile([64, H, SE], f32, name="seff")
        nc.vector.tensor_mul(
            seff[:], u_tiles[-1][:],
            brun[0:64, :, None].to_broadcast([64, H, SE]))

        cc_in = dram.tile([64, H * SE + H], f32, name="cc_in")
        cc_out = dram.tile([256, H * SE + H], f32, name="cc_out")
        nc.sync.dma_start(cc_in[:, 0:H * SE], seff.rearrange("p h e -> p (h e)"))
        nc.sync.dma_start(cc_in[:, H * SE:], brun[0:64, :])
        nc.gpsimd.collective_compute(
            "AllGather", ALU.bypass,
            replica_groups=[[0, 1, 2, 3], [4, 5, 6, 7]],
            ins=[cc_in.opt()], outs=[cc_out.opt()])

        sstart = keep.tile([64, H, SE], f32, name="sstart")
        nc.vector.tensor_copy(
            sstart.rearrange("p h e -> p (h e)"),
            nc.const_aps.aps[(f32, 0.0)][0:64].to_broadcast([64, H * SE]))
        for q in range(4):
            peer = keep.tile([64, H * SE + H], f32, name=f"peer{q}")
            nc.sync.dma_start(peer[:], cc_out[q * 64:(q + 1) * 64, :])
            aq = keep.tile([64, H], f32, name=f"aq{q}")
            # M = A_q*sel + (1-sel) ; S = S*M + sel*Seff_q
            nc.vector.tensor_mul(
                aq[:], peer[:, H * SE:],
                sel_sb[0:64, q:q + 1].to_broadcast([64, H]))
            nc.vector.tensor_add(
                aq[:], aq[:], isel_sb[0:64, q:q + 1].to_broadcast([64, H]))
            nc.vector.tensor_mul(
                sstart[:], sstart[:], aq[:, :, None].to_broadcast([64, H, SE]))
            psel = keep.tile([64, H, SE], f32, name=f"psel{q}")
            nc.vector.tensor_mul(
                psel.rearrange("p h e -> p (h e)"), peer[:, 0:H * SE],
                sel_sb[0:64, q:q + 1].to_broadcast([64, H * SE]))
            nc.vector.tensor_add(sstart[:], sstart[:], psel[:])

        # ---------------------------------------------------------- phase 2
        with contextlib.ExitStack() as ph2:
            wp2 = ph2.enter_context(tc.tile_pool(name="wp2", bufs=1))
            wk2 = ph2.enter_context(tc.tile_pool(name="wk2", bufs=2))
            ip = ph2.enter_context(tc.tile_pool(name="ip", bufs=2, space="PSUM"))
            tp2 = ph2.enter_context(tc.tile_pool(name="tp2", bufs=2, space="PSUM"))
            op = ph2.enter_context(tc.tile_pool(name="op", bufs=2, space="PSUM"))

            wo_sb = wp2.tile([128, KT, C], bf16, name="wo_sb")
            for kt in range(KT):
                nc.sync.dma_start(wo_sb[:, kt, :], P["wo"][:, kt, :])
            bo_sb = wp2.tile([1, C], f32, name="bo_sb")
            nc.sync.dma_start(bo_sb[:], P["bo2"][:])
            if has_ln:
                lnw_sb = wp2.tile([128, C], f32, name="lnw_sb")
                lnb_sb = wp2.tile([128, C], f32, name="lnb_sb")
                nc.sync.dma_start(lnw_sb[:], P["lnw"][:])
                nc.sync.dma_start(lnb_sb[:], P["lnb"][:])

            for ch in range(NCH):
                if ch == 0:
                    sacc = sstart
                else:
                    sacc = wk2.tile([64, H, SE], f32, name="sacc", tag="sacc")
                    nc.vector.tensor_add(sacc[:], sstart[:], u_tiles[ch - 1][:])
                sacb = wk2.tile([64, H, SE], bf16, name="sacb", tag="sacb")
                nc.vector.tensor_copy(sacb[:], sacc[:])

                attn = wk2.tile([128, H, D], f32, name="attn", tag="attn")
                den = wk2.tile([128, H], f32, name="den", tag="den")
                tnum = wk2.tile([128, H, SE], f32, name="tnum", tag="tnum")
                for g in range(4):
                    ips = ip.tile([128, 4, SE], f32, name="ips", tag="ips")
                    for j in range(4):
                        h = g * 4 + j
                        nc.tensor.matmul(ips[:, j, :], qt_tiles[ch][:, h, :],
                                         sacb[:, h, :],
                                         start=(j == 0), stop=(j == 3))
                    sl = slice(g * 4, (g + 1) * 4)
                    nc.vector.tensor_add(tnum[:, sl, :], ips[:],
                                         ni_tiles[ch][:, sl, :])
                    nc.vector.tensor_copy(den[:, sl], tnum[:, sl, D])
                nc.vector.tensor_scalar_max(den[:], den[:], EPS)
                nc.vector.reciprocal(den[:], den[:])
                nc.vector.tensor_mul(attn[:], tnum[:, :, 0:D],
                                     den[:, :, None].to_broadcast([128, H, D]))

                # transpose attn -> C-major, then o-proj (+bias) + residual
                at_sb = wk2.tile([128, KT, 128], bf16, name="at_sb", tag="at_sb")
                for tg in range(2):
                    tps = tp2.tile([128, 512], f32, name="tps2", tag="tps2")
                    for j in range(4):
                        nc.tensor.matmul(
                            tps[:, bass.ts(j, 128)],
                            attn.rearrange("p h d -> p (h d)")[
                                :, bass.ts(tg * 4 + j, 128)],
                            eyef_sb[:], is_transpose=True,
                            start=(j == 0), stop=(j == 3))
                    if tg == 0:
                        nc.vector.tensor_copy(
                            at_sb.rearrange("p k t -> p (k t)")[:, 0:512], tps[:])
                    else:
                        nc.scalar.copy(
                            at_sb.rearrange("p k t -> p (k t)")[:, 512:1024],
                            tps[:])

                ops = op.tile([128, C], f32, name="ops", tag="ops")
                for nh in range(2):
                    nsl = bass.ts(nh, 512)
                    for kt in range(KT):
                        nc.tensor.matmul(ops[:, nsl], at_sb[:, kt, :],
                                         wo_sb[:, kt, nsl],
                                         start=(kt == 0), stop=False)
                    nc.tensor.matmul(ops[:, nsl], ones_row[:],
                                     bo_sb[:, nsl], start=False, stop=True)

                xr = wk2.tile([128, C], f32, name="xr", tag="xr")
                nc.sync.dma_start(xr[:], P["xrows"][bass.ts(ch, 128), :])
                ysb = wk2.tile([128, C], f32, name="ysb", tag="ysb")
                nc.vector.tensor_add(ysb[:], ops[:], xr[:])

                # LayerNorm
                mus = wk2.tile([128, 4], f32, name="mus", tag="mus")
                scr = wk2.tile([128, C], f32, name="scr", tag="scr")
                nc.scalar.activation(scr[:], ysb[:], AF.Identity,
                                     accum_out=mus[:, 0:1])
                nc.vector.tensor_scalar_mul(mus[:, 1:2], mus[:, 0:1], -1.0 / C)
                nc.scalar.activation(scr[:], ysb[:], AF.Square,
                                     bias=mus[:, 1:2], accum_out=mus[:, 2:3])
                nc.vector.tensor_scalar(mus[:, 2:3], mus[:, 2:3], 1.0 / C,
                                        LN_EPS, ALU.mult, ALU.add)
                nc.scalar.activation(mus[:, 2:3], mus[:, 2:3], AF.Sqrt)
                nc.vector.reciprocal(mus[:, 2:3], mus[:, 2:3])
                nc.vector.tensor_mul(mus[:, 3:4], mus[:, 1:2], mus[:, 2:3])
                yln = wk2.tile([128, C], f32, name="yln", tag="yln")
                nc.scalar.activation(yln[:], ysb[:], AF.Identity,
                                     scale=mus[:, 2:3], bias=mus[:, 3:4])
                if has_ln:
                    nc.vector.tensor_mul(yln[:], yln[:], lnw_sb[:])
                    nc.vector.tensor_add(yln[:], yln[:], lnb_sb[:])
                nc.sync.dma_start(P["y"][bass.ts(ch, 128), :], yln[:])

    if SPLIT_WAITS:
        _split_multi_waits(nc)
    return nc


# ---------------------------------------------------------------- host side
def _rope_tables():
    half = D // 2
    inv = 1.0 / (ROPE_BASE ** (np.arange(half, dtype=np.float64) / half))
    t = np.arange(T, dtype=np.float64)
    fr = t[:, None] * inv[None, :]
    cos, sin = np.cos(fr), np.sin(fr)
    out = np.zeros((T, 128), np.float32)
    out[:, 0:32] = cos
    out[:, 32:64] = cos
    out[:, 64:96] = -sin
    out[:, 96:128] = sin
    return out


def _ktile(w, dtype=np.float32):  # [C, N] -> [128, KT, N]
    return np.ascontiguousarray(
        w.reshape(KT, 128, w.shape[1]).transpose(1, 0, 2)).astype(dtype)


_cache = {}
RUN_KW = {}      # extra kwargs for run_bass_kernel_spmd (test harness profiling)
LAST = None      # last BassKernelResults (test harness reads exec_time_ns)


def kernel(x, mask, Wq, Wk, Wv, Wg, bg, Wo, bo, ln_w, ln_b):
    bfl = ml_dtypes.bfloat16
    x = np.asarray(x, np.float32)
    mask = np.asarray(mask)
    has_mask = not np.all(mask == 1)
    has_ln = not (np.all(np.asarray(ln_w) == 1) and np.all(np.asarray(ln_b) == 0))

    key = (has_mask, has_ln)
    if key not in _cache:
        _cache[key] = build(has_mask, has_ln)
    nc = _cache[key]

    wqkv = _ktile(np.concatenate(
        [np.asarray(Wq).T, np.asarray(Wk).T, np.asarray(Wv).T], axis=1), bfl)
    wg = _ktile(np.ascontiguousarray(np.asarray(Wg, np.float32).T))
    wo_t = _ktile(np.ascontiguousarray(np.asarray(Wo).T), bfl)
    ropec_full = _rope_tables()
    triu = np.triu(np.ones((128, 128), np.float32))
    eye = np.eye(128)
    onesrow = np.ones((1, 128), np.float32)
    bgt = np.tile(np.asarray(bg, np.float32), (128, 1))
    bo2 = np.asarray(bo, np.float32)[None, :]

    in_maps = []
    for c in range(NCORE):
        b, r = c // 4, c % 4
        rows = slice(r * QT, (r + 1) * QT)
        xq = np.ascontiguousarray(x[b, rows].T)   # [C, QT]
        m = {
            "xT": _ktile(xq),
            "xTb": _ktile(xq, bfl),
            "wqkv": wqkv,
            "wg": wg,
            "wo": wo_t,
            "xrows": np.ascontiguousarray(x[b, rows]),
            "bo2": bo2,
            "ropec": np.ascontiguousarray(
                ropec_full[rows].reshape(NCH, 128, 128).transpose(1, 0, 2)),
            "triu": triu,
            "eye": eye.astype(bfl),
            "eyef": eye.astype(np.float32),
            "onesrow": onesrow,
            "bgt": bgt,
        }
        sel = np.zeros((128, 4), np.float32)
        sel[:, 0:r] = 1.0
        m["sel"] = sel
        m["isel"] = 1.0 - sel
        if has_mask:
            mk = np.asarray(mask[b, rows], np.float32)
            m["mkc"] = np.ascontiguousarray(mk.reshape(NCH, 128).T)
            m["mki"] = 1.0 - m["mkc"]
        if has_ln:
            m["lnw"] = np.tile(np.asarray(ln_w, np.float32), (128, 1))
            m["lnb"] = np.tile(np.asarray(ln_b, np.float32), (128, 1))
        in_maps.append(m)

    res = run_bass_kernel_spmd(nc, in_maps, list(range(NCORE)), **RUN_KW)
    globals()["LAST"] = res
    out = np.empty((B, T, C), np.float32)
    for c in range(NCORE):
        b, r = c // 4, c % 4
        out[b, r * QT:(r + 1) * QT, :] = res.results[c]["y"]
    return out
